# revision 5
# baseline (speedup 1.0000x reference)
"""Trainium2 Bass kernel for nn_LinearAttention (gated linear attention).

Math (per reference):
    qkv = x @ Wqkv.T ; q,k,v = split(qkv); q,k = elu(.)+1
    per (b,h): running_kv[t]  = d*running_kv[t-1]  + k[t]*v[t]   (elementwise, D=64)
               running_ksum[t]= d*running_ksum[t-1]+ k[t]
    den = clip(sum_d(q*running_ksum), 1e-6); out = q*running_kv/den
    g = sigmoid(out @ Wgate.T + bgate); out = g*out + (1-g)*v
    y = out @ Wout.T

Implementation strategy (8 NeuronCores, SPMD, no collectives):
  - Token-parallel: core c handles batch b=c//2, T-half h=c%2 (2048 tokens)
    plus a 512-token halo before the chunk to warm the decay scan
    (decay=0.95 => truncation error ~0.95^512 ~ 4e-12).  Half 0 gets a
    zero halo + k-mask so its scan state is exactly 0 at t=0.
  - Everything on-chip lives as [feature(partition), token(free)]; the host
    pre-transposes x and the weight matrices so both matmul operands are in
    natural layout and no on-chip transpose is ever needed.  The final
    output is produced transposed ([hidden, T]) and un-transposed on host.
  - The decay scan runs natively on the Vector engine via
    tensor_tensor_scan (state = d*state + u along the free/time axis),
    chained across 512-token groups via initial=prev[:, -1:].
  - den: sum over D=64 partitions via a 0/1 block-diagonal selector matmul
    (PSUM [16,512]); reciprocal broadcast back to 128 partitions via a
    second selector matmul in fp32r.
  - phi(x)=elu(x)+1 = exp(min(x,0)) + relu(x): DVE min, ACT Exp, then one
    fused scalar_tensor_tensor (max 0 then add).
  - bgate rides the Sigmoid drain as the ACT per-partition bias.
"""

import sys

for _p in ('/opt/trn_rl_repo', '/root/.axon_site'):
    if _p not in sys.path:
        sys.path.insert(0, _p)

from contextlib import ExitStack

import ml_dtypes
import numpy as np

import concourse.tile as tile
from concourse import bacc, mybir
from concourse.bass_utils import run_bass_kernel_spmd

F32 = mybir.dt.float32
BF16 = mybir.dt.bfloat16
AL = mybir.AluOpType
AF = mybir.ActivationFunctionType

B, T, HID = 4, 4096, 1024
H, D = 16, 64
OD = 3 * HID              # 3072 qkv output rows
NK = HID // 128           # 8 hidden (contraction) tiles
NOT = OD // 128           # 24 od tiles: q=0..7, k=8..15, v=16..23
HALF_T = T // 2           # 2048 tokens per core
HALO = 512
TLOC = HALO + HALF_T      # 2560
WG = 512                  # token-group width
NG = TLOC // WG           # 5 groups; group 0 = halo
NH = HID // 128           # 8 tiles per q/k/v section

_cache = {}


def _build_nc():
    nc = bacc.Bacc("TRN2", target_bir_lowering=False, debug=False)

    xT = nc.dram_tensor("xT", [HID, TLOC], BF16, kind="ExternalInput")
    wqkvT = nc.dram_tensor("wqkvT", [HID, OD], BF16, kind="ExternalInput")
    wgateT = nc.dram_tensor("wgateT", [HID, HID], BF16, kind="ExternalInput")
    woutT = nc.dram_tensor("woutT", [HID, HID], BF16, kind="ExternalInput")
    dec_c = nc.dram_tensor("dec_c", [128, NH], F32, kind="ExternalInput")
    mask_c = nc.dram_tensor("mask_c", [128, 1], F32, kind="ExternalInput")
    densel = nc.dram_tensor("densel", [128, NH * H], BF16, kind="ExternalInput")
    bcsel = nc.dram_tensor("bcsel", [H, NH * 128], mybir.dt.float32r, kind="ExternalInput")
    bgate_c = nc.dram_tensor("bgate_c", [128, NH], F32, kind="ExternalInput")
    yT = nc.dram_tensor("yT", [HID, HALF_T], F32, kind="ExternalOutput")

    with tile.TileContext(nc) as tc, ExitStack() as ctx:
        # ---- persistent pools -------------------------------------------
        consts = ctx.enter_context(tc.tile_pool(name="consts", bufs=1))
        wq_pool = ctx.enter_context(tc.tile_pool(name="wq", bufs=1))
        wg_pool = ctx.enter_context(tc.tile_pool(name="wgp", bufs=1))
        wo_pool = ctx.enter_context(tc.tile_pool(name="wop", bufs=1))
        xt_pool = ctx.enter_context(tc.tile_pool(name="xt", bufs=10))
        qkv_pool = ctx.enter_context(tc.tile_pool(name="qkv", bufs=9))
        tmp_pool = ctx.enter_context(tc.tile_pool(name="tmp", bufs=2))
        cum_pool = ctx.enter_context(tc.tile_pool(name="cum", bufs=1))
        st_pool = ctx.enter_context(tc.tile_pool(name="st", bufs=2))
        oa_pool = ctx.enter_context(tc.tile_pool(name="oa", bufs=9))
        gt_pool = ctx.enter_context(tc.tile_pool(name="gt", bufs=9))
        mix_pool = ctx.enter_context(tc.tile_pool(name="mix", bufs=9))
        y_pool = ctx.enter_context(tc.tile_pool(name="ysb", bufs=2))
        ps_pool = ctx.enter_context(tc.tile_pool(name="ps", bufs=7, space="PSUM"))
        psd_pool = ctx.enter_context(tc.tile_pool(name="psd", bufs=1, space="PSUM"))

        # ---- load constants / weights -----------------------------------
        wq_s = []
        for k in range(NK):
            w = wq_pool.tile([128, OD], BF16, tag=f"wq{k}")
            nc.sync.dma_start(w[:], wqkvT.ap()[128 * k:128 * (k + 1), :])
            wq_s.append(w)
        dec_s = consts.tile([128, NH], F32, tag="dec")
        nc.sync.dma_start(dec_s[:], dec_c.ap()[:, :])
        mask_s = consts.tile([128, 1], F32, tag="mask")
        nc.sync.dma_start(mask_s[:], mask_c.ap()[:, :])
        densel_s = consts.tile([128, NH * H], BF16, tag="densel")
        nc.sync.dma_start(densel_s[:], densel.ap()[:, :])
        bcsel_s = consts.tile([H, NH * 128], mybir.dt.float32r, tag="bcsel")
        nc.sync.dma_start(bcsel_s[:], bcsel.ap()[:, :])
        bgate_s = consts.tile([128, NH], F32, tag="bg")
        nc.sync.dma_start(bgate_s[:], bgate_c.ap()[:, :])
        wg_s, wo_s = [], []
        for k in range(NK):
            wgt = wg_pool.tile([128, HID], BF16, tag=f"wg{k}")
            nc.sync.dma_start(wgt[:], wgateT.ap()[128 * k:128 * (k + 1), :])
            wg_s.append(wgt)
            wot = wo_pool.tile([128, HID], BF16, tag=f"wo{k}")
            nc.sync.dma_start(wot[:], woutT.ap()[128 * k:128 * (k + 1), :])
            wo_s.append(wot)

        prev_kv = [None] * NH
        prev_ks = [None] * NH

        for g in range(NG):
            t0 = g * WG
            is_halo = g == 0
            tok = slice(t0, t0 + WG)

            # ---- load x tiles for this group ----------------------------
            xts = []
            for k in range(NK):
                xt_t = xt_pool.tile([128, WG], BF16, tag="xt")
                nc.sync.dma_start(xt_t[:], xT.ap()[128 * k:128 * (k + 1), tok])
                xts.append(xt_t)

            # ---- step 1: qkv projection (skip q tiles in halo group) ----
            q1 = [None] * NH
            k1 = [None] * NH
            vv = [None] * NH
            ots = list(range(NH, NOT)) if is_halo else list(range(NOT))
            for ot in ots:
                ps = ps_pool.tile([128, WG], F32, tag="mm")
                for k in range(NK):
                    nc.tensor.matmul(
                        ps[:], wq_s[k][:, 128 * ot:128 * (ot + 1)], xts[k][:],
                        start=(k == 0), stop=(k == NK - 1))
                if ot < NH:  # q
                    j = ot
                    qm = tmp_pool.tile([128, WG], BF16, tag="phim", bufs=3)
                    nc.vector.tensor_scalar_min(qm[:], ps[:], 0.0)
                    qe = tmp_pool.tile([128, WG], BF16, tag="phie", bufs=3)
                    nc.scalar.activation(qe[:], qm[:], AF.Exp)
                    q1[j] = qkv_pool.tile([128, WG], BF16, tag="q1", name=f"q1_{j}")
                    nc.vector.scalar_tensor_tensor(
                        q1[j][:], ps[:], 0.0, qe[:], AL.max, AL.add)
                elif ot < 2 * NH:  # k
                    j = ot - NH
                    km = tmp_pool.tile([128, WG], BF16, tag="phim", bufs=3)
                    nc.vector.tensor_scalar_min(km[:], ps[:], 0.0)
                    ke = tmp_pool.tile([128, WG], BF16, tag="phie", bufs=3)
                    nc.scalar.activation(ke[:], km[:], AF.Exp)
                    if is_halo:
                        kr = tmp_pool.tile([128, WG], BF16, tag="kraw")
                        nc.vector.scalar_tensor_tensor(
                            kr[:], ps[:], 0.0, ke[:], AL.max, AL.add)
                        k1[j] = qkv_pool.tile([128, WG], BF16, tag="k1", name=f"k1_{j}")
                        nc.vector.tensor_scalar_mul(
                            k1[j][:], kr[:], mask_s[:, 0:1])
                    else:
                        k1[j] = qkv_pool.tile([128, WG], BF16, tag="k1", name=f"k1_{j}")
                        nc.vector.scalar_tensor_tensor(
                            k1[j][:], ps[:], 0.0, ke[:], AL.max, AL.add)
                else:  # v
                    j = ot - 2 * NH
                    vv[j] = qkv_pool.tile([128, WG], BF16, tag="v", name=f"v_{j}", bufs=10)
                    nc.vector.tensor_copy(vv[j][:], ps[:])

            # ---- step 2+3: kv product and decay scans -------------------
            cum_kv = [None] * NH
            cum_ks = [None] * NH
            nkv = [None] * NH
            nks = [None] * NH
            for j in range(NH):
                kv = tmp_pool.tile([128, WG], BF16, tag="kvp", bufs=3)
                nc.vector.tensor_mul(kv[:], k1[j][:], vv[j][:])
                dec_b = dec_s[:, j:j + 1].broadcast_to([128, WG])
                cum_kv[j] = cum_pool.tile([128, WG], BF16, tag=f"ckv{j}", name=f"ckv_{j}")
                init_kv = 0.0 if g == 0 else prev_kv[j][:, 0:1]
                nc.vector.tensor_tensor_scan(
                    cum_kv[j][:], dec_b, kv[:], init_kv, AL.mult, AL.add)
                cum_ks[j] = cum_pool.tile([128, WG], BF16, tag=f"cks{j}", name=f"cks_{j}")
                init_ks = 0.0 if g == 0 else prev_ks[j][:, 0:1]
                nc.vector.tensor_tensor_scan(
                    cum_ks[j][:], dec_b, k1[j][:], init_ks, AL.mult, AL.add)
                if g < NG - 1:
                    nkv[j] = st_pool.tile([128, 1], F32, tag=f"skv{j}", name=f"skv_{j}")
                    nc.gpsimd.tensor_copy(nkv[j][:], cum_kv[j][:, WG - 1:WG])
                    nks[j] = st_pool.tile([128, 1], F32, tag=f"sks{j}", name=f"sks_{j}")
                    nc.gpsimd.tensor_copy(nks[j][:], cum_ks[j][:, WG - 1:WG])
            prev_kv = nkv
            prev_ks = nks

            if is_halo:
                continue

            # ---- step 4: denominator ------------------------------------
            dps = psd_pool.tile([H, WG], F32, tag="den")
            for j in range(NH):
                prod = tmp_pool.tile([128, WG], BF16, tag="prod", bufs=2)
                nc.vector.tensor_mul(prod[:], q1[j][:], cum_ks[j][:])
                nc.tensor.matmul(
                    dps[:], densel_s[:, H * j:H * (j + 1)], prod[:],
                    start=(j == 0), stop=(j == NH - 1))
            den_r = tmp_pool.tile([H, WG], F32, tag="denr")
            nc.vector.tensor_scalar_max(den_r[:], dps[:], 1e-6)
            den_i = tmp_pool.tile([H, WG], mybir.dt.float32r, tag="deni")
            with nc.allow_low_precision(reason="fp32r broadcast of reciprocal"):
                nc.vector.reciprocal(den_i[:], den_r[:])

            # ---- step 5: attention output -------------------------------
            oa = [None] * NH
            for j in range(NH):
                bc = ps_pool.tile([128, WG], F32, tag="mm")
                nc.tensor.matmul(
                    bc[:], bcsel_s[:, 128 * j:128 * (j + 1)],
                    den_i[:, :], start=True, stop=True)
                qkv_t = tmp_pool.tile([128, WG], BF16, tag="qckv")
                nc.vector.tensor_mul(qkv_t[:], q1[j][:], cum_kv[j][:])
                oa[j] = oa_pool.tile([128, WG], BF16, tag="oa", name=f"oa_{j}")
                nc.vector.tensor_mul(oa[j][:], qkv_t[:], bc[:])

            # ---- step 6: gate -------------------------------------------
            gt = [None] * NH
            for ot in range(NH):
                ps = ps_pool.tile([128, WG], F32, tag="mm")
                for k in range(NK):
                    nc.tensor.matmul(
                        ps[:], wg_s[k][:, 128 * ot:128 * (ot + 1)], oa[k][:],
                        start=(k == 0), stop=(k == NK - 1))
                gt[ot] = gt_pool.tile([128, WG], BF16, tag="gt", name=f"gt_{ot}")
                nc.scalar.activation(
                    gt[ot][:], ps[:], AF.Sigmoid, bias=bgate_s[:, ot:ot + 1])

            # ---- step 7: mix = v + g*(oa - v) ---------------------------
            mix = [None] * NH
            for j in range(NH):
                dl = tmp_pool.tile([128, WG], BF16, tag="dl")
                nc.vector.tensor_sub(dl[:], oa[j][:], vv[j][:])
                gd = tmp_pool.tile([128, WG], BF16, tag="gd")
                nc.vector.tensor_mul(gd[:], gt[j][:], dl[:])
                mix[j] = mix_pool.tile([128, WG], BF16, tag="mix", name=f"mix_{j}")
                nc.vector.tensor_add(mix[j][:], gd[:], vv[j][:])

            # ---- step 8: output projection ------------------------------
            out_tok = slice(t0 - HALO, t0 - HALO + WG)
            for ot in range(NH):
                ps = ps_pool.tile([128, WG], F32, tag="mm")
                for k in range(NK):
                    nc.tensor.matmul(
                        ps[:], wo_s[k][:, 128 * ot:128 * (ot + 1)], mix[k][:],
                        start=(k == 0), stop=(k == NK - 1))
                ysb = y_pool.tile([128, WG], F32, tag="ysb")
                nc.scalar.copy(ysb[:], ps[:])
                nc.sync.dma_start(
                    yT.ap()[128 * ot:128 * (ot + 1), out_tok], ysb[:])

    nc.compile()
    return nc


def _sigmoid(v):
    return 1.0 / (1.0 + np.exp(-v))


def _make_inputs(x, Wqkv, Wout, Wgate, bgate, decay_param):
    decay = _sigmoid(np.asarray(decay_param, np.float64)).astype(np.float32)
    bf = ml_dtypes.bfloat16
    wqkvT = np.ascontiguousarray(np.asarray(Wqkv, np.float32).T).astype(bf)
    wgateT = np.ascontiguousarray(np.asarray(Wgate, np.float32).T).astype(bf)
    woutT = np.ascontiguousarray(np.asarray(Wout, np.float32).T).astype(bf)

    p = np.arange(128)
    dec_c = np.empty((128, NH), np.float32)
    for j in range(NH):
        dec_c[:, j] = decay[2 * j + p // 64]
    densel = np.zeros((128, NH * H), np.float32)
    for j in range(NH):
        for pp in range(128):
            densel[pp, H * j + 2 * j + pp // 64] = 1.0
    bcsel = np.zeros((H, NH * 128), np.float32)
    for j in range(NH):
        for m in range(128):
            bcsel[2 * j + m // 64, 128 * j + m] = 1.0
    bgate_c = np.ascontiguousarray(
        np.asarray(bgate, np.float32).reshape(NH, 128).T)

    in_maps = []
    for c in range(8):
        b, half = c // 2, c % 2
        xb = np.asarray(x[b], np.float32)  # [T, HID]
        if half == 0:
            xloc = np.concatenate(
                [np.zeros((HALO, HID), np.float32), xb[:HALF_T]], axis=0)
            mask = np.zeros((128, 1), np.float32)
        else:
            xloc = xb[HALF_T - HALO:]
            mask = np.ones((128, 1), np.float32)
        in_maps.append({
            "xT": np.ascontiguousarray(xloc.T).astype(bf),
            "wqkvT": wqkvT, "wgateT": wgateT, "woutT": woutT,
            "dec_c": dec_c, "mask_c": mask,
            "densel": densel.astype(bf), "bcsel": bcsel,
            "bgate_c": bgate_c,
        })
    return in_maps


def kernel(x, Wqkv, Wout, Wgate, bgate, decay_param):
    if "nc" not in _cache:
        _cache["nc"] = _build_nc()
    nc = _cache["nc"]
    in_maps = _make_inputs(x, Wqkv, Wout, Wgate, bgate, decay_param)
    res = run_bass_kernel_spmd(nc, in_maps, list(range(8)))
    y = np.empty((B, T, HID), np.float32)
    for c in range(8):
        b, half = c // 2, c % 2
        y[b, half * HALF_T:(half + 1) * HALF_T, :] = res.results[c]["yT"].T
    return y
